# revision 22
# baseline (speedup 1.0000x reference)
"""Trainium2 Bass kernel for nn_MiddleFHD (sparse-aware implementation).

8 NeuronCores, H-sharded. L0/L1/L2 are computed as sparse im2col matmuls
over the active voxel set (1.2% occupancy); L3..L13 densely on per-core
H-slabs (channels on partitions, one accumulating matmul per conv tap via
AP-offset reads). BatchNorm batch-stats are computed as on-device per-core
partials; the host combines the 8 tiny [C,2] partials between launches and
the affine is applied on-device as relu(s*x+b) with per-partition vectors.
Empty/pad positions are filled with v = -(b+1)/s so relu(s*v+b) == 0.
"""

import sys

sys.path.insert(0, "/opt/trn_rl_repo")

import numpy as np
import ml_dtypes

import concourse.bacc as bacc
import concourse.mybir as mybir
from concourse import tile
from concourse.bass_utils import run_bass_kernel_spmd

F32 = mybir.dt.float32
BF16 = mybir.dt.bfloat16
NP_BF16 = ml_dtypes.bfloat16
NCORES = 8
EPS = 1e-5

D0, H0, W0 = 41, 200, 200
G2 = (21, 104, 100)   # valid h2 < 100
G5 = (11, 56, 50)     # valid h5 < 50
G9 = (5, 28, 25)      # valid h9 < 25
GF = (2, 28, 25)

TRUE_N = [41 * 200 * 200] * 2 + [21 * 100 * 100] * 3 + [11 * 50 * 50] * 4 \
    + [5 * 25 * 25] * 4 + [2 * 25 * 25]


def _ceil_to(x, m):
    return ((x + m - 1) // m) * m


def _taps(k):
    return [(a, b, c) for a in range(k[0]) for b in range(k[1])
            for c in range(k[2])]


# ---------------------------------------------------------------------------
# Host-side prep
# ---------------------------------------------------------------------------

class Prep:
    pass


def _dilate(occ, k, stride, pad, out_shape):
    Do, Ho, Wo = out_shape
    out = np.zeros(out_shape, bool)
    idx = np.argwhere(occ)
    for od in range(k[0]):
        for oh in range(k[1]):
            for ow in range(k[2]):
                t = idx + np.array([pad[0] - od, pad[1] - oh, pad[2] - ow])
                ok = ((t[:, 0] % stride[0] == 0) & (t[:, 1] % stride[1] == 0)
                      & (t[:, 2] % stride[2] == 0))
                t = t[ok] // np.array(stride)
                ok2 = ((t >= 0).all(1) & (t[:, 0] < Do) & (t[:, 1] < Ho)
                       & (t[:, 2] < Wo))
                t = t[ok2]
                out[t[:, 0], t[:, 1], t[:, 2]] = True
    return out


def prepare(coors):
    p = Prep()
    d = coors[:, 1].astype(np.int64)
    h = coors[:, 2].astype(np.int64)
    w = coors[:, 3].astype(np.int64)
    occ0 = np.zeros((D0, H0, W0), bool)
    occ0[d, h, w] = True

    order = np.lexsort((w, h, d))
    d, h, w = d[order], h[order], w[order]
    p.in_perm = order
    core_of = h // 26
    slots0 = []
    for k in range(NCORES):
        m = core_of == k
        slots0.append((d[m], h[m], w[m], np.nonzero(m)[0]))
    p.N0 = _ceil_to(max(512, max(len(s[0]) for s in slots0)), 512)
    p.slots0 = slots0

    gid0 = np.full((D0, H0, W0), -1, np.int64)
    for k in range(NCORES):
        dd, hh, ww, _ = slots0[k]
        gid0[dd, hh, ww] = k * p.N0 + np.arange(len(dd))

    taps27 = _taps((3, 3, 3))
    idx1 = np.full((27, NCORES, p.N0), -1, np.int64)
    for k in range(NCORES):
        dd, hh, ww, _ = slots0[k]
        n = len(dd)
        for t, (kd, kh, kw) in enumerate(taps27):
            sd_, sh_, sw_ = dd + kd - 1, hh + kh - 1, ww + kw - 1
            ok = ((sd_ >= 0) & (sd_ < D0) & (sh_ >= 0) & (sh_ < H0)
                  & (sw_ >= 0) & (sw_ < W0))
            v = np.full(n, -1, np.int64)
            v[ok] = gid0[sd_[ok], sh_[ok], sw_[ok]]
            idx1[t, k, :n] = v
    p.idx1 = idx1

    sup2v = _dilate(occ0, (3, 3, 3), (2, 2, 2), (1, 1, 1), (21, 100, 100))
    sup2 = np.zeros(G2, bool)
    sup2[:, :100, :] = sup2v
    p.sup2 = sup2
    d2, h2, w2 = np.nonzero(sup2)
    o = np.lexsort((w2, h2, d2))
    d2, h2, w2 = d2[o], h2[o], w2[o]
    core2 = h2 // 13
    slots2 = []
    for k in range(NCORES):
        m = core2 == k
        slots2.append((d2[m], h2[m], w2[m]))
    p.N2 = _ceil_to(max(512, max(len(s[0]) for s in slots2)), 512)
    p.slots2 = slots2

    idx2 = np.full((27, NCORES, p.N2), -1, np.int64)
    for k in range(NCORES):
        dd, hh, ww = slots2[k]
        n = len(dd)
        for t, (kd, kh, kw) in enumerate(taps27):
            sd_, sh_, sw_ = 2 * dd + kd - 1, 2 * hh + kh - 1, 2 * ww + kw - 1
            ok = ((sd_ >= 0) & (sd_ < D0) & (sh_ >= 0) & (sh_ < H0)
                  & (sw_ >= 0) & (sw_ < W0))
            v = np.full(n, -1, np.int64)
            v[ok] = gid0[sd_[ok], sh_[ok], sw_[ok]]
            idx2[t, k, :n] = v
    p.idx2 = idx2

    sup5v = _dilate(sup2[:, :100, :], (3, 3, 3), (2, 2, 2), (1, 1, 1),
                    (11, 50, 50))
    sup5 = np.zeros(G5, bool)
    sup5[:, :50, :] = sup5v
    p.sup5 = sup5
    sup9v = _dilate(sup5[:, :50, :], (3, 3, 3), (2, 2, 2), (0, 1, 1),
                    (5, 25, 25))
    sup9 = np.zeros(G9, bool)
    sup9[:, :25, :] = sup9v
    p.sup9 = sup9
    supF = np.zeros(GF, bool)
    supF[:, :25, :] = True
    p.supF = supF
    return p


# ---------------------------------------------------------------------------
# Dense specs
# ---------------------------------------------------------------------------

class DenseSpec:
    def __init__(self, li, Cin, Cout, ktaps, stride, in_grid, out_grid,
                 own_h, in_h0_of, in_hn, d_pad, w_pad=1):
        self.li = li
        self.Cin, self.Cout = Cin, Cout
        self.taps = ktaps
        self.sd, self.sh, self.sw = stride
        self.Din, self.Hin, self.Win = in_grid
        self.Dout, self.Hout, self.Wout = out_grid
        self.own_h = own_h
        self.in_h0_of = in_h0_of
        self.Hp = in_hn
        self.d_pad = d_pad
        self.w_pad = w_pad
        self.Wp = self.Wout * self.sw + 2 * w_pad
        self.G = 128 // Cin
        dstep = -(-self.Dout // self.G)
        self.chunks = [(g * dstep, min((g + 1) * dstep, self.Dout))
                       for g in range(self.G)]
        maxkd = max(t[0] for t in ktaps)

        def cnt(a, b):
            return (b - 1 - a) * self.sd + maxkd + 1

        # bf16 matmuls are ~4x faster on the PE (fp32 streams 4 cyc/col)
        # but ReLU threshold flips compound to ~2.5e-2 max-abs error across
        # the net; default to fp32 correctness, opt in via KERNEL_BF16=1.
        import os as _os
        self.mm_bf16 = (li <= 8) and _os.environ.get("KERNEL_BF16") == "1"
        self.use_stack = (3 <= li <= 8) and li != 5 and not self.mm_bf16
        self.bpst = 128 // Cin          # blocks per stack
        self.n_stacks = 8 // self.bpst  # full stacks holding blocks 0..7
        self.bake_last = self.use_stack and self.bpst >= 4
        self.n_wcols = self.n_stacks * 3 + (1 if self.bake_last else 0)
        self.Dxs = None
        self.Ys = None
        self.Dc = max(cnt(a, b) for a, b in self.chunks if b > a)
        self.Dxs = self.Dc - 2
        self.Ys = self.Hp - 2
        self.Lstk = self.Dxs * self.Ys * self.Wp
        self.Lc = self.Dc * self.Hp * self.Wp
        self.Lout = self.Dout * self.own_h * self.Wout
        T = max(1, 512 // self.Wout)
        self.htiles = []
        r = 0
        while r < self.own_h:
            t = min(T, self.own_h - r)
            self.htiles.append((r, t))
            r += t

    def slab_in_d0(self, g):
        return self.chunks[g][0] * self.sd - self.d_pad


def make_dense_specs():
    s = {}
    for li in (3, 4):
        s[li] = DenseSpec(li, 32, 32, _taps((3, 3, 3)), (1, 1, 1), G2, G2, 13,
                          lambda k: 13 * k - 1, 15, 1)
    s[5] = DenseSpec(5, 32, 64, _taps((3, 3, 3)), (2, 2, 2), G2, G5, 7,
                     lambda k: 14 * k - 1, 15, 1)
    for li in (6, 7, 8):
        s[li] = DenseSpec(li, 64, 64, _taps((3, 3, 3)), (1, 1, 1), G5, G5, 7,
                          lambda k: 7 * k - 1, 9, 1)
    s[9] = DenseSpec(9, 64, 64, _taps((3, 3, 3)), (2, 2, 2), G5, G9, 4,
                     lambda k: 8 * k - 1, 9, 0)
    for li in (10, 11, 12):
        s[li] = DenseSpec(li, 64, 64, _taps((3, 3, 3)), (1, 1, 1), G9, G9, 4,
                          lambda k: 4 * k - 1, 6, 1)
    s[13] = DenseSpec(13, 64, 64, _taps((3, 1, 1)), (2, 1, 1), G9, GF, 4,
                      lambda k: 4 * k, 4, 0, w_pad=0)
    return s


# ---------------------------------------------------------------------------
# Launch builders (compiled once per geometry)
# ---------------------------------------------------------------------------

_NC_CACHE = {}


def build_sparse_nc(Cout, N, apply_act):
    key = ("sparse", Cout, N, apply_act)
    if key in _NC_CACHE:
        return _NC_CACHE[key]
    nc = bacc.Bacc("TRN2", target_bir_lowering=False, debug=False,
                   num_devices=NCORES)
    imcol = nc.dram_tensor("imcol", [4, 128, N], F32, kind="ExternalInput")
    wsp = nc.dram_tensor("wsp", [4, 128, Cout], F32, kind="ExternalInput")
    sbv = (nc.dram_tensor("sbv", [4, 128, 2], F32, kind="ExternalInput")
           if apply_act else None)
    y_out = nc.dram_tensor("y", [Cout, N], F32, kind="ExternalOutput")
    part = nc.dram_tensor("part", [Cout, 2], F32, kind="ExternalOutput")

    BLK = 2048
    nblk = -(-N // BLK)
    ntiles_tot = N // 512

    with tile.TileContext(nc) as tc:
        with (tc.tile_pool(name="w", bufs=1) as wp,
              tc.tile_pool(name="sb", bufs=3) as sp,
              tc.tile_pool(name="ps", bufs=6, space="PSUM") as pp):
            wt = [wp.tile([128, Cout], F32, name=f"wt{c}", tag=f"wt{c}") for c in range(4)]
            for c in range(4):
                nc.sync.dma_start(wt[c][:], wsp[c])
            if apply_act:
                sbt = [wp.tile([128, 2], F32, name=f"sbt{c}", tag=f"sbt{c}") for c in range(4)]
                for c in range(4):
                    nc.sync.dma_start(sbt[c][:], sbv[c])
            acc1 = wp.tile([Cout, ntiles_tot], F32, tag="a1")
            acc2 = wp.tile([Cout, ntiles_tot], F32, tag="a2")
            gidx = 0
            for b in range(nblk):
                c0, c1 = b * BLK, min((b + 1) * BLK, N)
                W_ = c1 - c0
                im = [sp.tile([128, W_], F32, name=f"im{c}", tag=f"im{c}") for c in range(4)]
                for c in range(4):
                    nc.sync.dma_start(im[c][:], imcol[c, :, c0:c1])
                    if apply_act:
                        nc.scalar.activation(
                            im[c][:], im[c][:],
                            mybir.ActivationFunctionType.Relu,
                            bias=sbt[c][:, 1:2], scale=sbt[c][:, 0:1])
                yb = sp.tile([Cout, W_], F32, tag="yb")
                for t0 in range(0, W_, 512):
                    tw = min(512, W_ - t0)
                    ps = pp.tile([Cout, 512], F32, tag="ps")
                    for c in range(4):
                        nc.tensor.matmul(ps[:, :tw], wt[c][:],
                                         im[c][:, t0:t0 + tw],
                                         start=(c == 0), stop=(c == 3))
                    nc.scalar.copy(yb[:, t0:t0 + tw], ps[:, :tw])
                    scr = sp.tile([Cout, 512], F32, tag="scr")
                    nc.vector.reduce_sum(out=acc1[:, gidx:gidx + 1],
                                         in_=ps[:, :tw],
                                         axis=mybir.AxisListType.X)
                    nc.scalar.activation(scr[:, :tw], yb[:, t0:t0 + tw],
                                         mybir.ActivationFunctionType.Square,
                                         accum_out=acc2[:, gidx:gidx + 1])
                    gidx += 1
                nc.sync.dma_start(y_out[:, c0:c1], yb[:])
            pr = sp.tile([Cout, 2], F32, tag="pr")
            nc.vector.reduce_sum(out=pr[:, 0:1], in_=acc1[:],
                                 axis=mybir.AxisListType.X)
            nc.vector.reduce_sum(out=pr[:, 1:2], in_=acc2[:],
                                 axis=mybir.AxisListType.X)
            nc.sync.dma_start(part[:], pr[:])
    nc.compile()
    _NC_CACHE[key] = nc
    return nc


def build_dense_nc(spec):
    key = ("dense", spec.mm_bf16, spec.use_stack, spec.Cin, spec.Cout, spec.Lc, spec.Lout, spec.Dc,
           spec.Hp, spec.Wp, tuple(spec.taps), spec.sd, spec.sh, spec.sw,
           spec.Dout, spec.own_h, spec.Wout, tuple(spec.chunks))
    if key in _NC_CACHE:
        return _NC_CACHE[key]
    nc = bacc.Bacc("TRN2", target_bir_lowering=False, debug=False,
                   num_devices=NCORES)
    Cin, Cout, G = spec.Cin, spec.Cout, spec.G
    ntap = len(spec.taps)
    MDT = BF16 if spec.mm_bf16 else F32
    slab = nc.dram_tensor("slab", [128, spec.Lc], MDT, kind="ExternalInput")
    wd = nc.dram_tensor("wd", [128, ntap * Cout], MDT, kind="ExternalInput")
    sbv = nc.dram_tensor("sbv", [128, 2], F32, kind="ExternalInput")
    mchunk = spec.use_stack and spec.G * Cout == 128
    dstep = spec.chunks[0][1] - spec.chunks[0][0]
    if mchunk:
        maskd = nc.dram_tensor(
            "maskd", [128, dstep, spec.own_h, spec.Wout], F32,
            kind="ExternalInput")
    else:
        maskd = nc.dram_tensor(
            "maskd", [Cout, spec.Dout, spec.own_h, spec.Wout], F32,
            kind="ExternalInput")
    if spec.use_stack:
        wstk = nc.dram_tensor("wstk", [128, spec.n_wcols * Cout], F32,
                              kind="ExternalInput")
    y_out = nc.dram_tensor("y", [Cout, spec.Dout, spec.own_h, spec.Wout],
                           F32, kind="ExternalOutput")
    part = nc.dram_tensor("part", [Cout, 2], F32, kind="ExternalOutput")

    ntiles_tot = sum((b - a) for a, b in spec.chunks if b > a) \
        * len(spec.htiles)

    with tile.TileContext(nc) as tc:
        with (tc.tile_pool(name="w", bufs=1) as wp,
              tc.tile_pool(name="sb", bufs=6) as sp,
              tc.tile_pool(name="ps", bufs=8, space="PSUM") as pp):
            wt = wp.tile([128, ntap * Cout], MDT, tag="wt")
            nc.sync.dma_start(wt[:], wd[:])
            sbt = wp.tile([128, 2], F32, tag="sbt")
            nc.sync.dma_start(sbt[:], sbv[:])
            slab_t = wp.tile([128, spec.Lc], MDT, tag="slab")
            half = spec.Lc // 2
            nc.sync.dma_start(slab_t[:, :half], slab[:, :half])
            nc.sync.dma_start(slab_t[:, half:], slab[:, half:])
            nc.scalar.activation(slab_t[:], slab_t[:],
                                 mybir.ActivationFunctionType.Relu,
                                 bias=sbt[:, 1:2], scale=sbt[:, 0:1])
            if mchunk:
                mask_t = wp.tile([128, dstep, spec.own_h, spec.Wout], F32,
                                 tag="mask")
            else:
                mask_t = wp.tile([Cout, spec.Dout, spec.own_h, spec.Wout],
                                 F32, tag="mask")
            nc.sync.dma_start(mask_t[:], maskd[:])
            acc1 = wp.tile([Cout, ntiles_tot], F32, tag="a1")
            acc2 = wp.tile([Cout, ntiles_tot], F32, tag="a2")
            s4 = slab_t[:].rearrange("p (d h w) -> p d h w",
                                     d=spec.Dc, h=spec.Hp, w=spec.Wp)
            if spec.use_stack:
                wst = wp.tile([128, spec.n_wcols * Cout], F32, tag="wst")
                nc.sync.dma_start(wst[:], wstk[:])
            gidx = 0
            Lx = spec.Dxs * spec.Hp * spec.Wp
            HW_ = spec.Hp * spec.Wp

            def emit_tile(g, a, b, dt, h0, Th, stk4, dt_stk):
                ps = pp.tile([Cout, Th, spec.Wout], F32, name="ps", tag="ps")
                he = (Th - 1) * spec.sh + 1
                we = (spec.Wout - 1) * spec.sw + 1
                if spec.use_stack:
                    mi = 0
                    for s_ in range(spec.n_stacks):
                        for kw in range(3):
                            rhs = stk4[s_][:, dt_stk * spec.sd,
                                           h0 * spec.sh:
                                           h0 * spec.sh + he:spec.sh,
                                           kw:kw + we:spec.sw]
                            lhsT = wst[:, (s_ * 3 + kw) * Cout:
                                       (s_ * 3 + kw + 1) * Cout]
                            nc.tensor.matmul(ps[:], lhsT, rhs,
                                             start=(mi == 0), stop=False,
                                             tile_position=(0, 0))
                            mi += 1
                    if spec.bake_last:
                        rhs = stk4[spec.n_stacks][:, dt_stk * spec.sd,
                                                  h0 * spec.sh:
                                                  h0 * spec.sh + he:spec.sh,
                                                  0:we:spec.sw]
                        lhsT = wst[:, spec.n_stacks * 3 * Cout:
                                   (spec.n_stacks * 3 + 1) * Cout]
                        nc.tensor.matmul(ps[:], lhsT, rhs, start=False,
                                         stop=True, tile_position=(0, 0))
                    else:
                        for kw in range(3):
                            t = 24 + kw  # tap (kd=2, kh=2, kw)
                            rhs = s4[g * Cin:(g + 1) * Cin,
                                     dt * spec.sd + 2,
                                     h0 * spec.sh + 2:
                                     h0 * spec.sh + 2 + he:spec.sh,
                                     kw:kw + we:spec.sw]
                            lhsT = wt[g * Cin:(g + 1) * Cin,
                                      t * Cout:(t + 1) * Cout]
                            nc.tensor.matmul(ps[:], lhsT, rhs, start=False,
                                             stop=(kw == 2),
                                             tile_position=(g * Cin, 0))
                else:
                    for t, (kd, kh, kw) in enumerate(spec.taps):
                        rhs = s4[g * Cin:(g + 1) * Cin, dt * spec.sd + kd,
                                 h0 * spec.sh + kh:
                                 h0 * spec.sh + kh + he:spec.sh,
                                 kw:kw + we:spec.sw]
                        lhsT = wt[g * Cin:(g + 1) * Cin,
                                  t * Cout:(t + 1) * Cout]
                        nc.tensor.matmul(ps[:], lhsT, rhs, start=(t == 0),
                                         stop=(t == ntap - 1),
                                         tile_position=(g * Cin, 0))
                return ps

            with tc.tile_pool(name="stk", bufs=2) as stkp:
                for g in range(G):
                    a, b = spec.chunks[g]
                    if b <= a:
                        continue
                    # subchunk the d-range so double-buffered stacks fit SBUF
                    if spec.use_stack and Lx * 4 > 20000:
                        DS = 2 if spec.bake_last else -(-(b - a) // 2)
                    else:
                        DS = b - a
                    a2 = a
                    while a2 < b:
                        b2 = min(a2 + DS, b)
                        stk4 = None
                        if spec.use_stack:
                            Dxs2 = (b2 - a2 - 1) * spec.sd + 1
                            Lx2 = Dxs2 * HW_
                            nstk = spec.n_stacks + (1 if spec.bake_last
                                                    else 0)
                            stks = [stkp.tile([128, Lx2], F32,
                                              name=f"stk{s_}", tag=f"stk{s_}")
                                    for s_ in range(nstk)]
                            blocks = [(j // 3, j % 3, j // spec.bpst,
                                       j % spec.bpst, 0) for j in range(8)]
                            if spec.bake_last:
                                # baked (2,2,kw) blocks; slot 3 dup of kw=0
                                # keeps its rows finite under zero weights
                                for kw in range(3):
                                    blocks.append((2, 2, spec.n_stacks, kw,
                                                   kw))
                                blocks.append((2, 2, spec.n_stacks, 3, 0))
                            for (kd, kh, s_, slot, kwo) in blocks:
                                off = ((a2 - a) * spec.sd + kd) * HW_ \
                                    + kh * spec.Wp + kwo
                                lcp = min(Lx2, spec.Lc - off)
                                nc.sync.dma_start(
                                    stks[s_][slot * Cin:(slot + 1) * Cin,
                                             :lcp],
                                    slab_t[g * Cin:(g + 1) * Cin,
                                           off:off + lcp])
                            stk4 = [t_[:].rearrange(
                                "p (d h w) -> p d h w", d=Dxs2, h=spec.Hp,
                                w=spec.Wp) for t_ in stks]
                        for dt2 in range(b2 - a2):
                            dt = (a2 - a) + dt2
                            for (h0, Th) in spec.htiles:
                                ps = emit_tile(g, a, b, dt, h0, Th, stk4,
                                               dt2)
                                dg = a2 + dt2
                                yt = sp.tile([Cout, Th, spec.Wout], F32,
                                             name="yt", tag="yt")
                                if mchunk:
                                    msl = mask_t[g * Cout:(g + 1) * Cout, dt,
                                                 h0:h0 + Th, :]
                                else:
                                    msl = mask_t[:, dg, h0:h0 + Th, :]
                                nc.vector.tensor_mul(yt[:], ps[:], msl)
                                nc.vector.reduce_sum(
                                    out=acc1[:, gidx:gidx + 1], in_=yt[:],
                                    axis=mybir.AxisListType.XY)
                                scr = sp.tile([Cout, Th, spec.Wout], F32,
                                              name="scr", tag="scr")
                                nc.scalar.activation(
                                    scr[:], yt[:],
                                    mybir.ActivationFunctionType.Square,
                                    accum_out=acc2[:, gidx:gidx + 1])
                                nc.sync.dma_start(
                                    y_out[:, dg, h0:h0 + Th, :], yt[:])
                                gidx += 1
                        a2 = b2
            pr = sp.tile([Cout, 2], F32, tag="pr")
            nc.vector.reduce_sum(out=pr[:, 0:1], in_=acc1[:],
                                 axis=mybir.AxisListType.X)
            nc.vector.reduce_sum(out=pr[:, 1:2], in_=acc2[:],
                                 axis=mybir.AxisListType.X)
            nc.sync.dma_start(part[:], pr[:])
    nc.compile()
    _NC_CACHE[key] = nc
    return nc


def build_final_nc(L):
    key = ("final", L)
    if key in _NC_CACHE:
        return _NC_CACHE[key]
    nc = bacc.Bacc("TRN2", target_bir_lowering=False, debug=False,
                   num_devices=NCORES)
    y13 = nc.dram_tensor("y13", [64, L], F32, kind="ExternalInput")
    sbv = nc.dram_tensor("sbv", [64, 2], F32, kind="ExternalInput")
    maskd = nc.dram_tensor("maskd", [64, L], F32, kind="ExternalInput")
    out = nc.dram_tensor("out", [64, L], F32, kind="ExternalOutput")
    with tile.TileContext(nc) as tc:
        with tc.tile_pool(name="sb", bufs=2) as sp:
            yt = sp.tile([64, L], F32, tag="y")
            sbt = sp.tile([64, 2], F32, tag="sb")
            mt = sp.tile([64, L], F32, tag="m")
            nc.sync.dma_start(yt[:], y13[:])
            nc.sync.dma_start(sbt[:], sbv[:])
            nc.sync.dma_start(mt[:], maskd[:])
            nc.scalar.activation(yt[:], yt[:],
                                 mybir.ActivationFunctionType.Relu,
                                 bias=sbt[:, 1:2], scale=sbt[:, 0:1])
            ot = sp.tile([64, L], F32, tag="o")
            nc.vector.tensor_mul(ot[:], yt[:], mt[:])
            nc.sync.dma_start(out[:], ot[:])
    nc.compile()
    _NC_CACHE[key] = nc
    return nc


# ---------------------------------------------------------------------------
# Host glue
# ---------------------------------------------------------------------------

def _combine_stats(parts, gamma, beta, trueN):
    tot = np.sum(np.stack(parts), axis=0).astype(np.float64)
    mean = tot[:, 0] / trueN
    var = tot[:, 1] / trueN - mean * mean
    s = gamma / np.sqrt(var + EPS)
    b = beta - mean * s
    return s.astype(np.float32), b.astype(np.float32)


def _sparse_imcol(y_ext, idx, C):
    N = idx.shape[1]
    out = np.zeros((4, 128, N), np.float32)
    ii = np.where(idx < 0, y_ext.shape[1] - 1, idx)
    g = y_ext[:, ii]  # [C, 27, N]
    for t in range(27):
        out[t // 8, (t % 8) * C:(t % 8) * C + C, :] = g[:, t, :]
    return out


def _pack_sparse_w(w, Cout, Cin):
    out = np.zeros((4, 128, Cout), np.float32)
    t = 0
    for kd in range(3):
        for kh in range(3):
            for kw in range(3):
                out[t // 8, (t % 8) * Cin:(t % 8) * Cin + Cin, :] = \
                    w[:, :, kd, kh, kw].T
                t += 1
    return out


def _sb_sparse(s, b, Cin):
    out = np.zeros((4, 128, 2), np.float32)
    out[:, :, 0] = 1.0
    t = 0
    for t in range(27):
        r = (t % 8) * Cin
        out[t // 8, r:r + Cin, 0] = s
        out[t // 8, r:r + Cin, 1] = b
    return out


def _pack_dense_w(w, spec):
    ntap = len(spec.taps)
    out = np.zeros((128, ntap * spec.Cout),
                   NP_BF16 if spec.mm_bf16 else np.float32)
    for g in range(spec.G):
        for t, (kd, kh, kw) in enumerate(spec.taps):
            out[g * spec.Cin:(g + 1) * spec.Cin,
                t * spec.Cout:(t + 1) * spec.Cout] = w[:, :, kd, kh, kw].T
    return out


def _pack_stack_w(w, spec):
    # lhsT for stacked MMs: rows (slot, ci) over blocks 0..7, cols (s,kw,co)
    out = np.zeros((128, spec.n_wcols * spec.Cout), np.float32)
    taps9 = [(a, b) for a in range(3) for b in range(3)]
    for j in range(8):
        kd, kh = taps9[j]
        s_, slot = j // spec.bpst, j % spec.bpst
        for kw in range(3):
            col = (s_ * 3 + kw) * spec.Cout
            out[slot * spec.Cin:(slot + 1) * spec.Cin,
                col:col + spec.Cout] = w[:, :, kd, kh, kw].T
    if spec.bake_last:
        col = spec.n_stacks * 3 * spec.Cout
        for kw in range(3):
            out[kw * spec.Cin:(kw + 1) * spec.Cin,
                col:col + spec.Cout] = w[:, :, 2, 2, kw].T
    return out


def _dense_mask_chunked(sup, spec, k):
    h0 = k * spec.own_h
    dstep = spec.chunks[0][1] - spec.chunks[0][0]
    m = np.zeros((128, dstep, spec.own_h, spec.Wout), np.float32)
    hn = min(spec.own_h, sup.shape[1] - h0)
    for g in range(spec.G):
        a, b = spec.chunks[g]
        for dl in range(b - a):
            if hn > 0:
                m[g * spec.Cout:(g + 1) * spec.Cout, dl, :hn, :] =                     sup[a + dl, h0:h0 + hn, :]
    return m


def _sb_dense(s, b, spec):
    out = np.zeros((128, 2), np.float32)
    out[:, 0] = 1.0
    for g in range(spec.G):
        out[g * spec.Cin:(g + 1) * spec.Cin, 0] = s
        out[g * spec.Cin:(g + 1) * spec.Cin, 1] = b
    return out


def _dense_slab(y_dense, spec, k, v):
    """y_dense [C, Din, Hin, Win] -> per-core slab [128, Lc]."""
    C = spec.Cin
    sdt = NP_BF16 if spec.mm_bf16 else np.float32
    slab = np.empty((128, spec.Lc), sdt)
    slab[:] = np.tile(v, spec.G)[:, None].astype(sdt)
    s4 = slab.reshape(128, spec.Dc, spec.Hp, spec.Wp)
    h0 = spec.in_h0_of(k)
    hl0 = max(0, -h0)
    hl1 = min(spec.Hp, spec.Hin - h0)
    wp = spec.w_pad
    for g in range(spec.G):
        a, b = spec.chunks[g]
        if b <= a:
            continue
        d0 = spec.slab_in_d0(g)
        dl0 = max(0, -d0)
        dl1 = min(spec.Dc, spec.Din - d0)
        if dl1 <= dl0 or hl1 <= hl0:
            continue
        s4[g * C:(g + 1) * C, dl0:dl1, hl0:hl1, wp:wp + spec.Win] = \
            y_dense[:, d0 + dl0:d0 + dl1, h0 + hl0:h0 + hl1, :]
    return slab


def _dense_mask(sup, spec, k):
    h0 = k * spec.own_h
    m = np.zeros((spec.Dout, spec.own_h, spec.Wout), np.float32)
    hn = min(spec.own_h, sup.shape[1] - h0)
    if hn > 0:
        m[:, :hn, :] = sup[:, h0:h0 + hn, :]
    return np.broadcast_to(
        m[None], (spec.Cout, spec.Dout, spec.own_h, spec.Wout)).copy()


_SIM_CACHE = {}
SIM_TIMES = []


def _run(nc, in_maps):
    import os
    if os.environ.get("KERNEL_SIM_TIME") == "1":
        key = id(nc)
        if key not in _SIM_CACHE:
            from concourse.bass_interp import MultiCoreSim
            mcs = MultiCoreSim(nc, num_cores=1)
            core = mcs.cores[0]
            for name, val in in_maps[0].items():
                core.tensor(name)[:] = val
            mcs.simulate()
            _SIM_CACHE[key] = mcs.global_time
        SIM_TIMES.append(_SIM_CACHE[key])
    return run_bass_kernel_spmd(nc, in_maps,
                                core_ids=list(range(NCORES))).results


def _assemble_dense(outs, spec, sup, v):
    """Per-core dense outputs -> global grid with v at non-support."""
    C = spec.Cout
    yd = np.empty((C, spec.Dout, spec.Hout, spec.Wout), np.float32)
    yd[:] = v[:, None, None, None]
    for k in range(NCORES):
        h0 = k * spec.own_h
        hn = min(spec.own_h, spec.Hout - h0)
        if hn <= 0:
            continue
        vals = outs[k]["y"][:, :, :hn, :]
        m = sup[None, :, h0:h0 + hn, :]
        yd[:, :, h0:h0 + hn, :] = np.where(m, vals, v[:, None, None, None])
    return yd


DBG = {}
_PREP = None
LAST_HW_NS = 0
_STOP_AFTER = int(__import__("os").environ.get("KERNEL_STOP_AFTER", "99"))


def kernel(voxel_features, coors, batch_size, input_shape, conv_weights,
           gammas, betas):
    global _PREP
    voxel_features = np.asarray(voxel_features, np.float32)
    coors = np.asarray(coors)
    conv_weights = [np.asarray(w, np.float32) for w in conv_weights]
    gammas = [np.asarray(g, np.float32) for g in gammas]
    betas = [np.asarray(b, np.float32) for b in betas]

    p = prepare(coors)
    _PREP = p
    specs = make_dense_specs()
    feats = voxel_features[p.in_perm].T.astype(np.float32)  # [16, n] sorted

    # ---- L0 (sparse, raw input) ----
    f_ext = np.zeros((16, NCORES * p.N0 + 1), np.float32)
    for k in range(NCORES):
        pos = p.slots0[k][3]
        f_ext[:, k * p.N0:k * p.N0 + len(pos)] = feats[:, pos]
    nc0 = build_sparse_nc(16, p.N0, False)
    w0 = _pack_sparse_w(conv_weights[0], 16, 16)
    maps = [{"imcol": _sparse_imcol(f_ext, p.idx1[:, k, :], 16),
             "wsp": w0} for k in range(NCORES)]
    outs = _run(nc0, maps)
    s0, b0 = _combine_stats([o["part"] for o in outs], gammas[0], betas[0],
                            TRUE_N[0])
    y0 = [o["y"] for o in outs]
    DBG["y0"] = y0
    if _STOP_AFTER <= 0:
        return None

    # ---- L1 (sparse) ----
    v0 = (-(b0 + 1.0) / s0).astype(np.float32)
    y0_ext = np.concatenate(y0 + [v0[:, None]], axis=1)
    nc1 = build_sparse_nc(16, p.N0, True)
    w1 = _pack_sparse_w(conv_weights[1], 16, 16)
    sb1 = _sb_sparse(s0, b0, 16)
    maps = [{"imcol": _sparse_imcol(y0_ext, p.idx1[:, k, :], 16),
             "wsp": w1, "sbv": sb1} for k in range(NCORES)]
    outs = _run(nc1, maps)
    s1, b1 = _combine_stats([o["part"] for o in outs], gammas[1], betas[1],
                            TRUE_N[1])
    y1 = [o["y"] for o in outs]
    DBG["y1"] = y1
    if _STOP_AFTER <= 1:
        return None

    # ---- L2 (sparse, stride 2, 16->32) ----
    v1 = (-(b1 + 1.0) / s1).astype(np.float32)
    y1_ext = np.concatenate(y1 + [v1[:, None]], axis=1)
    nc2 = build_sparse_nc(32, p.N2, True)
    w2 = _pack_sparse_w(conv_weights[2], 32, 16)
    sb2 = _sb_sparse(s1, b1, 16)
    maps = [{"imcol": _sparse_imcol(y1_ext, p.idx2[:, k, :], 16),
             "wsp": w2, "sbv": sb2} for k in range(NCORES)]
    outs = _run(nc2, maps)
    s_prev, b_prev = _combine_stats([o["part"] for o in outs], gammas[2],
                                    betas[2], TRUE_N[2])
    # scatter L2 -> dense grid2 (fill v2)
    v2 = (-(b_prev + 1.0) / s_prev).astype(np.float32)
    y_dense = np.empty((32,) + G2, np.float32)
    y_dense[:] = v2[:, None, None, None]
    for k in range(NCORES):
        dd, hh, ww = p.slots2[k]
        y_dense[:, dd, hh, ww] = outs[k]["y"][:, :len(dd)]
    DBG["y2d"] = y_dense
    if _STOP_AFTER <= 2:
        return None

    # ---- dense layers ----
    sups = {3: p.sup2, 4: p.sup2, 5: p.sup5, 6: p.sup5, 7: p.sup5,
            8: p.sup5, 9: p.sup9, 10: p.sup9, 11: p.sup9, 12: p.sup9,
            13: p.supF}
    for li in range(3, 14):
        spec = specs[li]
        ncd = build_dense_nc(spec)
        wdp = _pack_dense_w(conv_weights[li], spec)
        sbd = _sb_dense(s_prev, b_prev, spec)
        v_prev = (-(b_prev + 1.0) / s_prev).astype(np.float32)
        mchunk = spec.use_stack and spec.G * spec.Cout == 128
        wstk_p = _pack_stack_w(conv_weights[li], spec) if spec.use_stack \
            else None
        maps = []
        for k in range(NCORES):
            mk = (_dense_mask_chunked(sups[li], spec, k) if mchunk
                  else _dense_mask(sups[li], spec, k))
            m = {"slab": _dense_slab(y_dense, spec, k, v_prev),
                 "wd": wdp, "sbv": sbd, "maskd": mk}
            if spec.use_stack:
                m["wstk"] = wstk_p
            maps.append(m)
        outs = _run(ncd, maps)
        s_prev, b_prev = _combine_stats([o["part"] for o in outs],
                                        gammas[li], betas[li], TRUE_N[li])
        v_new = (-(b_prev + 1.0) / s_prev).astype(np.float32)
        y_dense = _assemble_dense(outs, spec,
                                  sups[li][:spec.Dout, :spec.Hout, :],
                                  v_new)
        DBG[f"y{li}d"] = y_dense
        if _STOP_AFTER <= li:
            return None

    # ---- final BN+relu+mask for L13 ----
    # y_dense currently holds v at non-support; rebuild raw with 0 fill
    y13 = np.where(p.supF[None], y_dense, 0.0).astype(np.float32)
    L = 2 * 4 * 25
    ncf = build_final_nc(L)
    sbf = np.stack([s_prev, b_prev], axis=1).astype(np.float32)
    maps = []
    for k in range(NCORES):
        h0 = 4 * k
        sl = np.zeros((64, 2, 4, 25), np.float32)
        mk = np.zeros((64, 2, 4, 25), np.float32)
        hn = min(4, 28 - h0)
        sl[:, :, :hn, :] = y13[:, :, h0:h0 + hn, :]
        vh = max(0, min(4, 25 - h0))
        mk[:, :, :vh, :] = 1.0
        maps.append({"y13": sl.reshape(64, L), "sbv": sbf,
                     "maskd": mk.reshape(64, L)})
    outs = _run(ncf, maps)
    global LAST_HW_NS
    LAST_HW_NS = sum(SIM_TIMES)
    SIM_TIMES.clear()
    final = np.zeros((64, 2, 25, 25), np.float32)
    for k in range(NCORES):
        h0 = 4 * k
        hn = max(0, min(4, 25 - h0))
        if hn:
            final[:, :, h0:h0 + hn, :] = \
                outs[k]["out"].reshape(64, 2, 4, 25)[:, :, :hn, :]
    return final.reshape(1, 128, 25, 25)


# revision 23
# speedup vs baseline: 1.1726x; 1.1726x over previous
"""Trainium2 Bass kernel for nn_MiddleFHD (sparse-aware implementation).

8 NeuronCores, H-sharded. L0/L1/L2 are computed as sparse im2col matmuls
over the active voxel set (1.2% occupancy); L3..L13 densely on per-core
H-slabs (channels on partitions, one accumulating matmul per conv tap via
AP-offset reads). BatchNorm batch-stats are computed as on-device per-core
partials; the host combines the 8 tiny [C,2] partials between launches and
the affine is applied on-device as relu(s*x+b) with per-partition vectors.
Empty/pad positions are filled with v = -(b+1)/s so relu(s*v+b) == 0.
"""

import sys

sys.path.insert(0, "/opt/trn_rl_repo")

import numpy as np
import ml_dtypes

import concourse.bacc as bacc
import concourse.mybir as mybir
from concourse import tile
from concourse.bass_utils import run_bass_kernel_spmd

F32 = mybir.dt.float32
BF16 = mybir.dt.bfloat16
NP_BF16 = ml_dtypes.bfloat16
NCORES = 8
EPS = 1e-5

D0, H0, W0 = 41, 200, 200
G2 = (21, 104, 100)   # valid h2 < 100
G5 = (11, 56, 50)     # valid h5 < 50
G9 = (5, 28, 25)      # valid h9 < 25
GF = (2, 28, 25)

TRUE_N = [41 * 200 * 200] * 2 + [21 * 100 * 100] * 3 + [11 * 50 * 50] * 4 \
    + [5 * 25 * 25] * 4 + [2 * 25 * 25]


def _ceil_to(x, m):
    return ((x + m - 1) // m) * m


def _taps(k):
    return [(a, b, c) for a in range(k[0]) for b in range(k[1])
            for c in range(k[2])]


# ---------------------------------------------------------------------------
# Host-side prep
# ---------------------------------------------------------------------------

class Prep:
    pass


def _dilate(occ, k, stride, pad, out_shape):
    Do, Ho, Wo = out_shape
    out = np.zeros(out_shape, bool)
    idx = np.argwhere(occ)
    for od in range(k[0]):
        for oh in range(k[1]):
            for ow in range(k[2]):
                t = idx + np.array([pad[0] - od, pad[1] - oh, pad[2] - ow])
                ok = ((t[:, 0] % stride[0] == 0) & (t[:, 1] % stride[1] == 0)
                      & (t[:, 2] % stride[2] == 0))
                t = t[ok] // np.array(stride)
                ok2 = ((t >= 0).all(1) & (t[:, 0] < Do) & (t[:, 1] < Ho)
                       & (t[:, 2] < Wo))
                t = t[ok2]
                out[t[:, 0], t[:, 1], t[:, 2]] = True
    return out


def prepare(coors):
    p = Prep()
    d = coors[:, 1].astype(np.int64)
    h = coors[:, 2].astype(np.int64)
    w = coors[:, 3].astype(np.int64)
    occ0 = np.zeros((D0, H0, W0), bool)
    occ0[d, h, w] = True

    order = np.lexsort((w, h, d))
    d, h, w = d[order], h[order], w[order]
    p.in_perm = order
    core_of = h // 26
    slots0 = []
    for k in range(NCORES):
        m = core_of == k
        slots0.append((d[m], h[m], w[m], np.nonzero(m)[0]))
    p.N0 = _ceil_to(max(512, max(len(s[0]) for s in slots0)), 512)
    p.slots0 = slots0

    gid0 = np.full((D0, H0, W0), -1, np.int64)
    for k in range(NCORES):
        dd, hh, ww, _ = slots0[k]
        gid0[dd, hh, ww] = k * p.N0 + np.arange(len(dd))

    taps27 = _taps((3, 3, 3))
    idx1 = np.full((27, NCORES, p.N0), -1, np.int64)
    for k in range(NCORES):
        dd, hh, ww, _ = slots0[k]
        n = len(dd)
        for t, (kd, kh, kw) in enumerate(taps27):
            sd_, sh_, sw_ = dd + kd - 1, hh + kh - 1, ww + kw - 1
            ok = ((sd_ >= 0) & (sd_ < D0) & (sh_ >= 0) & (sh_ < H0)
                  & (sw_ >= 0) & (sw_ < W0))
            v = np.full(n, -1, np.int64)
            v[ok] = gid0[sd_[ok], sh_[ok], sw_[ok]]
            idx1[t, k, :n] = v
    p.idx1 = idx1

    sup2v = _dilate(occ0, (3, 3, 3), (2, 2, 2), (1, 1, 1), (21, 100, 100))
    sup2 = np.zeros(G2, bool)
    sup2[:, :100, :] = sup2v
    p.sup2 = sup2
    d2, h2, w2 = np.nonzero(sup2)
    o = np.lexsort((w2, h2, d2))
    d2, h2, w2 = d2[o], h2[o], w2[o]
    core2 = h2 // 13
    slots2 = []
    for k in range(NCORES):
        m = core2 == k
        slots2.append((d2[m], h2[m], w2[m]))
    p.N2 = _ceil_to(max(512, max(len(s[0]) for s in slots2)), 512)
    p.slots2 = slots2

    idx2 = np.full((27, NCORES, p.N2), -1, np.int64)
    for k in range(NCORES):
        dd, hh, ww = slots2[k]
        n = len(dd)
        for t, (kd, kh, kw) in enumerate(taps27):
            sd_, sh_, sw_ = 2 * dd + kd - 1, 2 * hh + kh - 1, 2 * ww + kw - 1
            ok = ((sd_ >= 0) & (sd_ < D0) & (sh_ >= 0) & (sh_ < H0)
                  & (sw_ >= 0) & (sw_ < W0))
            v = np.full(n, -1, np.int64)
            v[ok] = gid0[sd_[ok], sh_[ok], sw_[ok]]
            idx2[t, k, :n] = v
    p.idx2 = idx2

    sup5v = _dilate(sup2[:, :100, :], (3, 3, 3), (2, 2, 2), (1, 1, 1),
                    (11, 50, 50))
    sup5 = np.zeros(G5, bool)
    sup5[:, :50, :] = sup5v
    p.sup5 = sup5
    sup9v = _dilate(sup5[:, :50, :], (3, 3, 3), (2, 2, 2), (0, 1, 1),
                    (5, 25, 25))
    sup9 = np.zeros(G9, bool)
    sup9[:, :25, :] = sup9v
    p.sup9 = sup9
    supF = np.zeros(GF, bool)
    supF[:, :25, :] = True
    p.supF = supF
    return p


# ---------------------------------------------------------------------------
# Dense specs
# ---------------------------------------------------------------------------

class DenseSpec:
    def __init__(self, li, Cin, Cout, ktaps, stride, in_grid, out_grid,
                 own_h, in_h0_of, in_hn, d_pad, w_pad=1):
        self.li = li
        self.Cin, self.Cout = Cin, Cout
        self.taps = ktaps
        self.sd, self.sh, self.sw = stride
        self.Din, self.Hin, self.Win = in_grid
        self.Dout, self.Hout, self.Wout = out_grid
        self.own_h = own_h
        self.in_h0_of = in_h0_of
        self.Hp = in_hn
        self.d_pad = d_pad
        self.w_pad = w_pad
        self.Wp = self.Wout * self.sw + 2 * w_pad
        self.G = 128 // Cin
        dstep = -(-self.Dout // self.G)
        self.chunks = [(g * dstep, min((g + 1) * dstep, self.Dout))
                       for g in range(self.G)]
        maxkd = max(t[0] for t in ktaps)

        def cnt(a, b):
            return (b - 1 - a) * self.sd + maxkd + 1

        # bf16 matmuls are ~4x faster on the PE (fp32 streams 4 cyc/col)
        # but ReLU threshold flips compound to ~2.5e-2 max-abs error across
        # the net; default to fp32 correctness, opt in via KERNEL_BF16=1.
        import os as _os
        self.mm_bf16 = (li <= 8) and _os.environ.get("KERNEL_BF16") == "1"
        self.use_stack = (3 <= li <= 8) and li != 5 and not self.mm_bf16
        self.bpst = 128 // Cin          # blocks per stack
        self.n_stacks = 8 // self.bpst  # full stacks holding blocks 0..7
        self.Dxs = None
        self.Ys = None
        self.Dc = max(cnt(a, b) for a, b in self.chunks if b > a)
        self.Dxs = self.Dc - 2
        self.Ys = self.Hp - 2
        self.Lstk = self.Dxs * self.Ys * self.Wp
        self.Lc = self.Dc * self.Hp * self.Wp
        self.Lout = self.Dout * self.own_h * self.Wout
        T = max(1, 512 // self.Wout)
        self.htiles = []
        r = 0
        while r < self.own_h:
            t = min(T, self.own_h - r)
            self.htiles.append((r, t))
            r += t

    def slab_in_d0(self, g):
        return self.chunks[g][0] * self.sd - self.d_pad


def make_dense_specs():
    s = {}
    for li in (3, 4):
        s[li] = DenseSpec(li, 32, 32, _taps((3, 3, 3)), (1, 1, 1), G2, G2, 13,
                          lambda k: 13 * k - 1, 15, 1)
    s[5] = DenseSpec(5, 32, 64, _taps((3, 3, 3)), (2, 2, 2), G2, G5, 7,
                     lambda k: 14 * k - 1, 15, 1)
    for li in (6, 7, 8):
        s[li] = DenseSpec(li, 64, 64, _taps((3, 3, 3)), (1, 1, 1), G5, G5, 7,
                          lambda k: 7 * k - 1, 9, 1)
    s[9] = DenseSpec(9, 64, 64, _taps((3, 3, 3)), (2, 2, 2), G5, G9, 4,
                     lambda k: 8 * k - 1, 9, 0)
    for li in (10, 11, 12):
        s[li] = DenseSpec(li, 64, 64, _taps((3, 3, 3)), (1, 1, 1), G9, G9, 4,
                          lambda k: 4 * k - 1, 6, 1)
    s[13] = DenseSpec(13, 64, 64, _taps((3, 1, 1)), (2, 1, 1), G9, GF, 4,
                      lambda k: 4 * k, 4, 0, w_pad=0)
    return s


# ---------------------------------------------------------------------------
# Launch builders (compiled once per geometry)
# ---------------------------------------------------------------------------

_NC_CACHE = {}


def build_sparse_nc(Cout, N, apply_act):
    key = ("sparse", Cout, N, apply_act)
    if key in _NC_CACHE:
        return _NC_CACHE[key]
    nc = bacc.Bacc("TRN2", target_bir_lowering=False, debug=False,
                   num_devices=NCORES)
    imcol = nc.dram_tensor("imcol", [4, 128, N], F32, kind="ExternalInput")
    wsp = nc.dram_tensor("wsp", [4, 128, Cout], F32, kind="ExternalInput")
    sbv = (nc.dram_tensor("sbv", [4, 128, 2], F32, kind="ExternalInput")
           if apply_act else None)
    y_out = nc.dram_tensor("y", [Cout, N], F32, kind="ExternalOutput")
    part = nc.dram_tensor("part", [Cout, 2], F32, kind="ExternalOutput")

    BLK = 2048
    nblk = -(-N // BLK)
    ntiles_tot = N // 512

    with tile.TileContext(nc) as tc:
        with (tc.tile_pool(name="w", bufs=1) as wp,
              tc.tile_pool(name="sb", bufs=3) as sp,
              tc.tile_pool(name="ps", bufs=6, space="PSUM") as pp):
            wt = [wp.tile([128, Cout], F32, name=f"wt{c}", tag=f"wt{c}") for c in range(4)]
            for c in range(4):
                nc.sync.dma_start(wt[c][:], wsp[c])
            if apply_act:
                sbt = [wp.tile([128, 2], F32, name=f"sbt{c}", tag=f"sbt{c}") for c in range(4)]
                for c in range(4):
                    nc.sync.dma_start(sbt[c][:], sbv[c])
            acc1 = wp.tile([Cout, ntiles_tot], F32, tag="a1")
            acc2 = wp.tile([Cout, ntiles_tot], F32, tag="a2")
            gidx = 0
            for b in range(nblk):
                c0, c1 = b * BLK, min((b + 1) * BLK, N)
                W_ = c1 - c0
                im = [sp.tile([128, W_], F32, name=f"im{c}", tag=f"im{c}") for c in range(4)]
                for c in range(4):
                    nc.sync.dma_start(im[c][:], imcol[c, :, c0:c1])
                    if apply_act:
                        nc.scalar.activation(
                            im[c][:], im[c][:],
                            mybir.ActivationFunctionType.Relu,
                            bias=sbt[c][:, 1:2], scale=sbt[c][:, 0:1])
                yb = sp.tile([Cout, W_], F32, tag="yb")
                for t0 in range(0, W_, 512):
                    tw = min(512, W_ - t0)
                    ps = pp.tile([Cout, 512], F32, tag="ps")
                    for c in range(4):
                        nc.tensor.matmul(ps[:, :tw], wt[c][:],
                                         im[c][:, t0:t0 + tw],
                                         start=(c == 0), stop=(c == 3))
                    nc.scalar.copy(yb[:, t0:t0 + tw], ps[:, :tw])
                    scr = sp.tile([Cout, 512], F32, tag="scr")
                    nc.vector.reduce_sum(out=acc1[:, gidx:gidx + 1],
                                         in_=ps[:, :tw],
                                         axis=mybir.AxisListType.X)
                    nc.scalar.activation(scr[:, :tw], yb[:, t0:t0 + tw],
                                         mybir.ActivationFunctionType.Square,
                                         accum_out=acc2[:, gidx:gidx + 1])
                    gidx += 1
                nc.sync.dma_start(y_out[:, c0:c1], yb[:])
            pr = sp.tile([Cout, 2], F32, tag="pr")
            nc.vector.reduce_sum(out=pr[:, 0:1], in_=acc1[:],
                                 axis=mybir.AxisListType.X)
            nc.vector.reduce_sum(out=pr[:, 1:2], in_=acc2[:],
                                 axis=mybir.AxisListType.X)
            nc.sync.dma_start(part[:], pr[:])
    nc.compile()
    _NC_CACHE[key] = nc
    return nc


def build_dense_nc(spec):
    key = ("dense", spec.mm_bf16, spec.use_stack, spec.Cin, spec.Cout, spec.Lc, spec.Lout, spec.Dc,
           spec.Hp, spec.Wp, tuple(spec.taps), spec.sd, spec.sh, spec.sw,
           spec.Dout, spec.own_h, spec.Wout, tuple(spec.chunks))
    if key in _NC_CACHE:
        return _NC_CACHE[key]
    nc = bacc.Bacc("TRN2", target_bir_lowering=False, debug=False,
                   num_devices=NCORES)
    Cin, Cout, G = spec.Cin, spec.Cout, spec.G
    ntap = len(spec.taps)
    MDT = BF16 if spec.mm_bf16 else F32
    slab = nc.dram_tensor("slab", [128, spec.Lc], MDT, kind="ExternalInput")
    wd = nc.dram_tensor("wd", [128, ntap * Cout], MDT, kind="ExternalInput")
    sbv = nc.dram_tensor("sbv", [128, 2], F32, kind="ExternalInput")
    mchunk = spec.use_stack and spec.G * Cout == 128
    dstep = spec.chunks[0][1] - spec.chunks[0][0]
    if mchunk:
        maskd = nc.dram_tensor(
            "maskd", [128, dstep, spec.own_h, spec.Wout], F32,
            kind="ExternalInput")
    else:
        maskd = nc.dram_tensor(
            "maskd", [Cout, spec.Dout, spec.own_h, spec.Wout], F32,
            kind="ExternalInput")
    if spec.use_stack:
        wstk = nc.dram_tensor("wstk", [128, spec.n_stacks * 3 * Cout], F32,
                              kind="ExternalInput")
    y_out = nc.dram_tensor("y", [Cout, spec.Dout, spec.own_h, spec.Wout],
                           F32, kind="ExternalOutput")
    part = nc.dram_tensor("part", [Cout, 2], F32, kind="ExternalOutput")

    ntiles_tot = sum((b - a) for a, b in spec.chunks if b > a) \
        * len(spec.htiles)

    with tile.TileContext(nc) as tc:
        with (tc.tile_pool(name="w", bufs=1) as wp,
              tc.tile_pool(name="sb", bufs=6) as sp,
              tc.tile_pool(name="ps", bufs=8, space="PSUM") as pp):
            wt = wp.tile([128, ntap * Cout], MDT, tag="wt")
            nc.sync.dma_start(wt[:], wd[:])
            sbt = wp.tile([128, 2], F32, tag="sbt")
            nc.sync.dma_start(sbt[:], sbv[:])
            slab_t = wp.tile([128, spec.Lc], MDT, tag="slab")
            half = spec.Lc // 2
            nc.sync.dma_start(slab_t[:, :half], slab[:, :half])
            nc.sync.dma_start(slab_t[:, half:], slab[:, half:])
            nc.scalar.activation(slab_t[:], slab_t[:],
                                 mybir.ActivationFunctionType.Relu,
                                 bias=sbt[:, 1:2], scale=sbt[:, 0:1])
            if mchunk:
                mask_t = wp.tile([128, dstep, spec.own_h, spec.Wout], F32,
                                 tag="mask")
            else:
                mask_t = wp.tile([Cout, spec.Dout, spec.own_h, spec.Wout],
                                 F32, tag="mask")
            nc.sync.dma_start(mask_t[:], maskd[:])
            acc1 = wp.tile([Cout, ntiles_tot], F32, tag="a1")
            acc2 = wp.tile([Cout, ntiles_tot], F32, tag="a2")
            s4 = slab_t[:].rearrange("p (d h w) -> p d h w",
                                     d=spec.Dc, h=spec.Hp, w=spec.Wp)
            if spec.use_stack:
                wst = wp.tile([128, spec.n_stacks * 3 * Cout], F32, tag="wst")
                nc.sync.dma_start(wst[:], wstk[:])
            gidx = 0
            Lx = spec.Dxs * spec.Hp * spec.Wp
            HW_ = spec.Hp * spec.Wp

            def emit_tile(g, a, b, dt, h0, Th, stk4, dt_stk):
                ps = pp.tile([Cout, Th, spec.Wout], F32, name="ps", tag="ps")
                he = (Th - 1) * spec.sh + 1
                we = (spec.Wout - 1) * spec.sw + 1
                if spec.use_stack:
                    mi = 0
                    for s_ in range(spec.n_stacks):
                        for kw in range(3):
                            rhs = stk4[s_][:, dt_stk * spec.sd,
                                           h0 * spec.sh:
                                           h0 * spec.sh + he:spec.sh,
                                           kw:kw + we:spec.sw]
                            lhsT = wst[:, (s_ * 3 + kw) * Cout:
                                       (s_ * 3 + kw + 1) * Cout]
                            nc.tensor.matmul(ps[:], lhsT, rhs,
                                             start=(mi == 0), stop=False,
                                             tile_position=(0, 0))
                            mi += 1
                    for kw in range(3):
                        t = 24 + kw  # tap (kd=2, kh=2, kw)
                        rhs = s4[g * Cin:(g + 1) * Cin, dt * spec.sd + 2,
                                 h0 * spec.sh + 2:h0 * spec.sh + 2 + he:
                                 spec.sh,
                                 kw:kw + we:spec.sw]
                        lhsT = wt[g * Cin:(g + 1) * Cin,
                                  t * Cout:(t + 1) * Cout]
                        nc.tensor.matmul(ps[:], lhsT, rhs, start=False,
                                         stop=(kw == 2),
                                         tile_position=(g * Cin, 0))
                else:
                    for t, (kd, kh, kw) in enumerate(spec.taps):
                        rhs = s4[g * Cin:(g + 1) * Cin, dt * spec.sd + kd,
                                 h0 * spec.sh + kh:
                                 h0 * spec.sh + kh + he:spec.sh,
                                 kw:kw + we:spec.sw]
                        lhsT = wt[g * Cin:(g + 1) * Cin,
                                  t * Cout:(t + 1) * Cout]
                        nc.tensor.matmul(ps[:], lhsT, rhs, start=(t == 0),
                                         stop=(t == ntap - 1),
                                         tile_position=(g * Cin, 0))
                return ps

            with tc.tile_pool(name="stk", bufs=2) as stkp:
                for g in range(G):
                    a, b = spec.chunks[g]
                    if b <= a:
                        continue
                    # subchunk the d-range so double-buffered stacks fit SBUF
                    if spec.use_stack and Lx * 4 > 20000:
                        DS = -(-(b - a) // 2)
                    else:
                        DS = b - a
                    a2 = a
                    while a2 < b:
                        b2 = min(a2 + DS, b)
                        stk4 = None
                        if spec.use_stack:
                            Dxs2 = (b2 - a2 - 1) * spec.sd + 1
                            Lx2 = Dxs2 * HW_
                            stks = [stkp.tile([128, Lx2], F32,
                                              name=f"stk{s_}", tag=f"stk{s_}")
                                    for s_ in range(spec.n_stacks)]
                            for j in range(8):
                                kd, kh = j // 3, j % 3
                                s_, slot = j // spec.bpst, j % spec.bpst
                                off = ((a2 - a) * spec.sd + kd) * HW_ \
                                    + kh * spec.Wp
                                lcp = min(Lx2, spec.Lc - off)
                                nc.sync.dma_start(
                                    stks[s_][slot * Cin:(slot + 1) * Cin,
                                             :lcp],
                                    slab_t[g * Cin:(g + 1) * Cin,
                                           off:off + lcp])
                            stk4 = [t_[:].rearrange(
                                "p (d h w) -> p d h w", d=Dxs2, h=spec.Hp,
                                w=spec.Wp) for t_ in stks]
                        for dt2 in range(b2 - a2):
                            dt = (a2 - a) + dt2
                            for (h0, Th) in spec.htiles:
                                ps = emit_tile(g, a, b, dt, h0, Th, stk4,
                                               dt2)
                                dg = a2 + dt2
                                yt = sp.tile([Cout, Th, spec.Wout], F32,
                                             name="yt", tag="yt")
                                if mchunk:
                                    msl = mask_t[g * Cout:(g + 1) * Cout, dt,
                                                 h0:h0 + Th, :]
                                else:
                                    msl = mask_t[:, dg, h0:h0 + Th, :]
                                nc.vector.tensor_mul(yt[:], ps[:], msl)
                                nc.vector.reduce_sum(
                                    out=acc1[:, gidx:gidx + 1], in_=yt[:],
                                    axis=mybir.AxisListType.XY)
                                scr = sp.tile([Cout, Th, spec.Wout], F32,
                                              name="scr", tag="scr")
                                nc.scalar.activation(
                                    scr[:], yt[:],
                                    mybir.ActivationFunctionType.Square,
                                    accum_out=acc2[:, gidx:gidx + 1])
                                nc.sync.dma_start(
                                    y_out[:, dg, h0:h0 + Th, :], yt[:])
                                gidx += 1
                        a2 = b2
            pr = sp.tile([Cout, 2], F32, tag="pr")
            nc.vector.reduce_sum(out=pr[:, 0:1], in_=acc1[:],
                                 axis=mybir.AxisListType.X)
            nc.vector.reduce_sum(out=pr[:, 1:2], in_=acc2[:],
                                 axis=mybir.AxisListType.X)
            nc.sync.dma_start(part[:], pr[:])
    nc.compile()
    _NC_CACHE[key] = nc
    return nc


def build_final_nc(L):
    key = ("final", L)
    if key in _NC_CACHE:
        return _NC_CACHE[key]
    nc = bacc.Bacc("TRN2", target_bir_lowering=False, debug=False,
                   num_devices=NCORES)
    y13 = nc.dram_tensor("y13", [64, L], F32, kind="ExternalInput")
    sbv = nc.dram_tensor("sbv", [64, 2], F32, kind="ExternalInput")
    maskd = nc.dram_tensor("maskd", [64, L], F32, kind="ExternalInput")
    out = nc.dram_tensor("out", [64, L], F32, kind="ExternalOutput")
    with tile.TileContext(nc) as tc:
        with tc.tile_pool(name="sb", bufs=2) as sp:
            yt = sp.tile([64, L], F32, tag="y")
            sbt = sp.tile([64, 2], F32, tag="sb")
            mt = sp.tile([64, L], F32, tag="m")
            nc.sync.dma_start(yt[:], y13[:])
            nc.sync.dma_start(sbt[:], sbv[:])
            nc.sync.dma_start(mt[:], maskd[:])
            nc.scalar.activation(yt[:], yt[:],
                                 mybir.ActivationFunctionType.Relu,
                                 bias=sbt[:, 1:2], scale=sbt[:, 0:1])
            ot = sp.tile([64, L], F32, tag="o")
            nc.vector.tensor_mul(ot[:], yt[:], mt[:])
            nc.sync.dma_start(out[:], ot[:])
    nc.compile()
    _NC_CACHE[key] = nc
    return nc


# ---------------------------------------------------------------------------
# Host glue
# ---------------------------------------------------------------------------

def _combine_stats(parts, gamma, beta, trueN):
    tot = np.sum(np.stack(parts), axis=0).astype(np.float64)
    mean = tot[:, 0] / trueN
    var = tot[:, 1] / trueN - mean * mean
    s = gamma / np.sqrt(var + EPS)
    b = beta - mean * s
    return s.astype(np.float32), b.astype(np.float32)


def _sparse_imcol(y_ext, idx, C):
    N = idx.shape[1]
    out = np.zeros((4, 128, N), np.float32)
    ii = np.where(idx < 0, y_ext.shape[1] - 1, idx)
    g = y_ext[:, ii]  # [C, 27, N]
    for t in range(27):
        out[t // 8, (t % 8) * C:(t % 8) * C + C, :] = g[:, t, :]
    return out


def _pack_sparse_w(w, Cout, Cin):
    out = np.zeros((4, 128, Cout), np.float32)
    t = 0
    for kd in range(3):
        for kh in range(3):
            for kw in range(3):
                out[t // 8, (t % 8) * Cin:(t % 8) * Cin + Cin, :] = \
                    w[:, :, kd, kh, kw].T
                t += 1
    return out


def _sb_sparse(s, b, Cin):
    out = np.zeros((4, 128, 2), np.float32)
    out[:, :, 0] = 1.0
    t = 0
    for t in range(27):
        r = (t % 8) * Cin
        out[t // 8, r:r + Cin, 0] = s
        out[t // 8, r:r + Cin, 1] = b
    return out


def _pack_dense_w(w, spec):
    ntap = len(spec.taps)
    out = np.zeros((128, ntap * spec.Cout),
                   NP_BF16 if spec.mm_bf16 else np.float32)
    for g in range(spec.G):
        for t, (kd, kh, kw) in enumerate(spec.taps):
            out[g * spec.Cin:(g + 1) * spec.Cin,
                t * spec.Cout:(t + 1) * spec.Cout] = w[:, :, kd, kh, kw].T
    return out


def _pack_stack_w(w, spec):
    # lhsT for stacked MMs: rows (slot, ci) over blocks 0..7, cols (s,kw,co)
    out = np.zeros((128, spec.n_stacks * 3 * spec.Cout), np.float32)
    taps9 = [(a, b) for a in range(3) for b in range(3)]
    for j in range(8):
        kd, kh = taps9[j]
        s_, slot = j // spec.bpst, j % spec.bpst
        for kw in range(3):
            col = (s_ * 3 + kw) * spec.Cout
            out[slot * spec.Cin:(slot + 1) * spec.Cin,
                col:col + spec.Cout] = w[:, :, kd, kh, kw].T
    return out


def _dense_mask_chunked(sup, spec, k):
    h0 = k * spec.own_h
    dstep = spec.chunks[0][1] - spec.chunks[0][0]
    m = np.zeros((128, dstep, spec.own_h, spec.Wout), np.float32)
    hn = min(spec.own_h, sup.shape[1] - h0)
    for g in range(spec.G):
        a, b = spec.chunks[g]
        for dl in range(b - a):
            if hn > 0:
                m[g * spec.Cout:(g + 1) * spec.Cout, dl, :hn, :] =                     sup[a + dl, h0:h0 + hn, :]
    return m


def _sb_dense(s, b, spec):
    out = np.zeros((128, 2), np.float32)
    out[:, 0] = 1.0
    for g in range(spec.G):
        out[g * spec.Cin:(g + 1) * spec.Cin, 0] = s
        out[g * spec.Cin:(g + 1) * spec.Cin, 1] = b
    return out


def _dense_slab(y_dense, spec, k, v):
    """y_dense [C, Din, Hin, Win] -> per-core slab [128, Lc]."""
    C = spec.Cin
    sdt = NP_BF16 if spec.mm_bf16 else np.float32
    slab = np.empty((128, spec.Lc), sdt)
    slab[:] = np.tile(v, spec.G)[:, None].astype(sdt)
    s4 = slab.reshape(128, spec.Dc, spec.Hp, spec.Wp)
    h0 = spec.in_h0_of(k)
    hl0 = max(0, -h0)
    hl1 = min(spec.Hp, spec.Hin - h0)
    wp = spec.w_pad
    for g in range(spec.G):
        a, b = spec.chunks[g]
        if b <= a:
            continue
        d0 = spec.slab_in_d0(g)
        dl0 = max(0, -d0)
        dl1 = min(spec.Dc, spec.Din - d0)
        if dl1 <= dl0 or hl1 <= hl0:
            continue
        s4[g * C:(g + 1) * C, dl0:dl1, hl0:hl1, wp:wp + spec.Win] = \
            y_dense[:, d0 + dl0:d0 + dl1, h0 + hl0:h0 + hl1, :]
    return slab


def _dense_mask(sup, spec, k):
    h0 = k * spec.own_h
    m = np.zeros((spec.Dout, spec.own_h, spec.Wout), np.float32)
    hn = min(spec.own_h, sup.shape[1] - h0)
    if hn > 0:
        m[:, :hn, :] = sup[:, h0:h0 + hn, :]
    return np.broadcast_to(
        m[None], (spec.Cout, spec.Dout, spec.own_h, spec.Wout)).copy()


_SIM_CACHE = {}
SIM_TIMES = []


def _run(nc, in_maps):
    import os
    if os.environ.get("KERNEL_SIM_TIME") == "1":
        key = id(nc)
        if key not in _SIM_CACHE:
            from concourse.bass_interp import MultiCoreSim
            mcs = MultiCoreSim(nc, num_cores=1)
            core = mcs.cores[0]
            for name, val in in_maps[0].items():
                core.tensor(name)[:] = val
            mcs.simulate()
            _SIM_CACHE[key] = mcs.global_time
        SIM_TIMES.append(_SIM_CACHE[key])
    return run_bass_kernel_spmd(nc, in_maps,
                                core_ids=list(range(NCORES))).results


def _assemble_dense(outs, spec, sup, v):
    """Per-core dense outputs -> global grid with v at non-support."""
    C = spec.Cout
    yd = np.empty((C, spec.Dout, spec.Hout, spec.Wout), np.float32)
    yd[:] = v[:, None, None, None]
    for k in range(NCORES):
        h0 = k * spec.own_h
        hn = min(spec.own_h, spec.Hout - h0)
        if hn <= 0:
            continue
        vals = outs[k]["y"][:, :, :hn, :]
        m = sup[None, :, h0:h0 + hn, :]
        yd[:, :, h0:h0 + hn, :] = np.where(m, vals, v[:, None, None, None])
    return yd


DBG = {}
_PREP = None
LAST_HW_NS = 0
_STOP_AFTER = int(__import__("os").environ.get("KERNEL_STOP_AFTER", "99"))


def kernel(voxel_features, coors, batch_size, input_shape, conv_weights,
           gammas, betas):
    global _PREP
    voxel_features = np.asarray(voxel_features, np.float32)
    coors = np.asarray(coors)
    conv_weights = [np.asarray(w, np.float32) for w in conv_weights]
    gammas = [np.asarray(g, np.float32) for g in gammas]
    betas = [np.asarray(b, np.float32) for b in betas]

    p = prepare(coors)
    _PREP = p
    specs = make_dense_specs()
    feats = voxel_features[p.in_perm].T.astype(np.float32)  # [16, n] sorted

    # ---- L0 (sparse, raw input) ----
    f_ext = np.zeros((16, NCORES * p.N0 + 1), np.float32)
    for k in range(NCORES):
        pos = p.slots0[k][3]
        f_ext[:, k * p.N0:k * p.N0 + len(pos)] = feats[:, pos]
    nc0 = build_sparse_nc(16, p.N0, False)
    w0 = _pack_sparse_w(conv_weights[0], 16, 16)
    maps = [{"imcol": _sparse_imcol(f_ext, p.idx1[:, k, :], 16),
             "wsp": w0} for k in range(NCORES)]
    outs = _run(nc0, maps)
    s0, b0 = _combine_stats([o["part"] for o in outs], gammas[0], betas[0],
                            TRUE_N[0])
    y0 = [o["y"] for o in outs]
    DBG["y0"] = y0
    if _STOP_AFTER <= 0:
        return None

    # ---- L1 (sparse) ----
    v0 = (-(b0 + 1.0) / s0).astype(np.float32)
    y0_ext = np.concatenate(y0 + [v0[:, None]], axis=1)
    nc1 = build_sparse_nc(16, p.N0, True)
    w1 = _pack_sparse_w(conv_weights[1], 16, 16)
    sb1 = _sb_sparse(s0, b0, 16)
    maps = [{"imcol": _sparse_imcol(y0_ext, p.idx1[:, k, :], 16),
             "wsp": w1, "sbv": sb1} for k in range(NCORES)]
    outs = _run(nc1, maps)
    s1, b1 = _combine_stats([o["part"] for o in outs], gammas[1], betas[1],
                            TRUE_N[1])
    y1 = [o["y"] for o in outs]
    DBG["y1"] = y1
    if _STOP_AFTER <= 1:
        return None

    # ---- L2 (sparse, stride 2, 16->32) ----
    v1 = (-(b1 + 1.0) / s1).astype(np.float32)
    y1_ext = np.concatenate(y1 + [v1[:, None]], axis=1)
    nc2 = build_sparse_nc(32, p.N2, True)
    w2 = _pack_sparse_w(conv_weights[2], 32, 16)
    sb2 = _sb_sparse(s1, b1, 16)
    maps = [{"imcol": _sparse_imcol(y1_ext, p.idx2[:, k, :], 16),
             "wsp": w2, "sbv": sb2} for k in range(NCORES)]
    outs = _run(nc2, maps)
    s_prev, b_prev = _combine_stats([o["part"] for o in outs], gammas[2],
                                    betas[2], TRUE_N[2])
    # scatter L2 -> dense grid2 (fill v2)
    v2 = (-(b_prev + 1.0) / s_prev).astype(np.float32)
    y_dense = np.empty((32,) + G2, np.float32)
    y_dense[:] = v2[:, None, None, None]
    for k in range(NCORES):
        dd, hh, ww = p.slots2[k]
        y_dense[:, dd, hh, ww] = outs[k]["y"][:, :len(dd)]
    DBG["y2d"] = y_dense
    if _STOP_AFTER <= 2:
        return None

    # ---- dense layers ----
    sups = {3: p.sup2, 4: p.sup2, 5: p.sup5, 6: p.sup5, 7: p.sup5,
            8: p.sup5, 9: p.sup9, 10: p.sup9, 11: p.sup9, 12: p.sup9,
            13: p.supF}
    for li in range(3, 14):
        spec = specs[li]
        ncd = build_dense_nc(spec)
        wdp = _pack_dense_w(conv_weights[li], spec)
        sbd = _sb_dense(s_prev, b_prev, spec)
        v_prev = (-(b_prev + 1.0) / s_prev).astype(np.float32)
        mchunk = spec.use_stack and spec.G * spec.Cout == 128
        wstk_p = _pack_stack_w(conv_weights[li], spec) if spec.use_stack \
            else None
        maps = []
        for k in range(NCORES):
            mk = (_dense_mask_chunked(sups[li], spec, k) if mchunk
                  else _dense_mask(sups[li], spec, k))
            m = {"slab": _dense_slab(y_dense, spec, k, v_prev),
                 "wd": wdp, "sbv": sbd, "maskd": mk}
            if spec.use_stack:
                m["wstk"] = wstk_p
            maps.append(m)
        outs = _run(ncd, maps)
        s_prev, b_prev = _combine_stats([o["part"] for o in outs],
                                        gammas[li], betas[li], TRUE_N[li])
        v_new = (-(b_prev + 1.0) / s_prev).astype(np.float32)
        y_dense = _assemble_dense(outs, spec,
                                  sups[li][:spec.Dout, :spec.Hout, :],
                                  v_new)
        DBG[f"y{li}d"] = y_dense
        if _STOP_AFTER <= li:
            return None

    # ---- final BN+relu+mask for L13 ----
    # y_dense currently holds v at non-support; rebuild raw with 0 fill
    y13 = np.where(p.supF[None], y_dense, 0.0).astype(np.float32)
    L = 2 * 4 * 25
    ncf = build_final_nc(L)
    sbf = np.stack([s_prev, b_prev], axis=1).astype(np.float32)
    maps = []
    for k in range(NCORES):
        h0 = 4 * k
        sl = np.zeros((64, 2, 4, 25), np.float32)
        mk = np.zeros((64, 2, 4, 25), np.float32)
        hn = min(4, 28 - h0)
        sl[:, :, :hn, :] = y13[:, :, h0:h0 + hn, :]
        vh = max(0, min(4, 25 - h0))
        mk[:, :, :vh, :] = 1.0
        maps.append({"y13": sl.reshape(64, L), "sbv": sbf,
                     "maskd": mk.reshape(64, L)})
    outs = _run(ncf, maps)
    global LAST_HW_NS
    LAST_HW_NS = sum(SIM_TIMES)
    SIM_TIMES.clear()
    final = np.zeros((64, 2, 25, 25), np.float32)
    for k in range(NCORES):
        h0 = 4 * k
        hn = max(0, min(4, 25 - h0))
        if hn:
            final[:, :, h0:h0 + hn, :] = \
                outs[k]["out"].reshape(64, 2, 4, 25)[:, :, :hn, :]
    return final.reshape(1, 128, 25, 25)
